# revision 37
# baseline (speedup 1.0000x reference)
"""Distributed MultiHeadAttention kernel for 8 TRN2 NeuronCores.

Problem: B=4, S=2048, E=1024, H=16 heads of dim 64, causal attention.
Sharding: core i handles (batch b = i//2, head-group hg = i%2) -> 8 heads.
Each core computes qkv for its heads, causal attention, and a partial
output projection over its heads' features; the host sums the two
partials per batch and adds the bias.

Layout notes (per core):
  xT      (1024, 2048) bf16 : x[b].T               (e on partitions)
  wqkT    (1024, 1024) bf16 : per-pair [qA|qB|kA|kB] blocks of 128 cols,
                              q rows pre-scaled by HD**-0.5
  wvT     (1024, 512)  bf16 : v weights, head-major (h*64+d)
  wprojT  (512, 1024)  bf16 : rows c=(pair, head-in-pair, d), cols e
  masks   (4, 128, 512) bf16: causal step masks (only [0] used)
  yT out  (1024, 2048) f32  : partial (W_proj @ attn.T), pre-bias

Inputs are DMA'd interleaved (wv, x s-slice 0, wqk, x s-slices 1-3,
wproj) so the V and QKV accumulation chains start on the first chunks;
V-phase psum chains rotate over all three PSUM pools during the stream.

On-chip pipeline per pair p (heads 2p, 2p+1 packed at partitions 0:64 /
64:128 for K=64 tensor-engine row-group pairing):
  qkT tiles (128, 2048) fp16 produced by bf16 QKV matmuls (DVE/ACT copy),
  scoresT (k-part, q-free) (128, 2x512) blocks in one 2-bank PSUM tensor
  -> single wide exp (ACT, bf16 out), diagonal blocks trimmed to the
     valid causal span; the leading (128,128) triangle masked via
     gpsimd affine_select (or DVE mask-multiply)
  -> PV with per-head [V|1] bf16 stationary -> rows 0:65 of a (128, 512)
     PSUM accum (row 64 = softmax denominator)
  -> per-(qc,head) reciprocal (rec= act | lnexp | vec | fast) + gpsimd
     partition_broadcast (or PE rank-1 matmul into rows 64:128 of the
     same bank) + DVE multiply -> normalized attnT bf16
  proj: wprojT bf16 stationary x attnT -> yT partial blocks (f32 out).
"""

import numpy as np
import ml_dtypes

import concourse.bass as bass
import concourse.mybir as mybir
import concourse.tile as tile
from concourse import bacc
from concourse.alu_op_type import AluOpType

F32 = mybir.dt.float32
F32R = mybir.dt.float32r
BF16 = mybir.dt.bfloat16
F16 = mybir.dt.float16
I32 = mybir.dt.int32
AF = mybir.ActivationFunctionType

# magic constant for the fast-inverse seed y0 = bits^-1(C - bits(x))
RECIP_MAGIC = 0x7EF311C3

B, S, E, H = 4, 2048, 1024, 16
HD = 64
HC = 8           # heads per core
NPAIR = 4        # head pairs per core
EC = E // 128    # 8 e-chunks
QC = S // 512    # 4 q-chunks
KB = S // 128    # 16 k-blocks
ST = S // 128    # 16 s-tiles
VW = HC * (HD + 1)  # 520: v features + per-head ones column


def _act_raw(nc, out, in_, func):
    eng = nc.scalar
    inputs = [eng.lower_ap(in_)]
    for val in (0.0, 1.0, 0.0):  # bias, scale, alpha
        inputs.append(mybir.ImmediateValue(dtype=mybir.dt.float32, value=val))
    return eng.add_instruction(
        mybir.InstActivation(
            name=nc.get_next_instruction_name(),
            func=func,
            ins=inputs,
            outs=[eng.lower_ap(out)],
        )
    )


def build_nc(repeats=1, qk_dtype=F16, low_dt=BF16, probs_bufs=8,
             qk_bufs=4, ycopy="dve", rec="nr", qkcopy="dve", maskeng="pool",
             xw_dtype=BF16, mm_bufs=2, pv_bufs=2, small_bufs=4, bcast="pool"):
    nc = bacc.Bacc("TRN2", target_bir_lowering=False, debug=False)
    xT = nc.dram_tensor("xT", (E, S), xw_dtype, kind="ExternalInput")
    wqkT = nc.dram_tensor("wqkT", (E, HC * 128), xw_dtype, kind="ExternalInput")
    wvT = nc.dram_tensor("wvT", (E, HC * HD), xw_dtype, kind="ExternalInput")
    wprojT = nc.dram_tensor("wprojT", (HC * HD, E), low_dt, kind="ExternalInput")
    masks = nc.dram_tensor("masks", (4, 128, 512), low_dt, kind="ExternalInput")
    yT = nc.dram_tensor("yT", (E, S), F32, kind="ExternalOutput")

    with tile.TileContext(nc) as tc:
        for _rep in range(repeats):
            _emit_body(nc, tc, xT, wqkT, wvT, wprojT, masks, yT,
                       qk_dtype=qk_dtype, low_dt=low_dt,
                       probs_bufs=probs_bufs, qk_bufs=qk_bufs, ycopy=ycopy,
                       rec=rec, qkcopy=qkcopy, maskeng=maskeng,
                       xw_dtype=xw_dtype, mm_bufs=mm_bufs, pv_bufs=pv_bufs,
                       small_bufs=small_bufs, bcast=bcast)
    nc.compile()
    return nc


def _emit_body(nc, tc, xT, wqkT, wvT, wprojT, masks, yT, qk_dtype=F16,
               low_dt=BF16, probs_bufs=8, qk_bufs=4, ycopy="dve",
               rec="nr", qkcopy="dve", maskeng="pool", xw_dtype=BF16,
               mm_bufs=2, pv_bufs=2, small_bufs=4, bcast="pool"):
    if True:
        with tc.tile_pool(name="vp", bufs=1) as v_pool, \
             tc.tile_pool(name="qk", bufs=qk_bufs) as qk_pool, \
             tc.tile_pool(name="probs", bufs=probs_bufs) as probs_pool, \
             tc.tile_pool(name="attn", bufs=1) as attn_pool, \
             tc.tile_pool(name="small", bufs=small_bufs) as small_pool, \
             tc.tile_pool(name="mm", bufs=mm_bufs, space="PSUM") as mm_ps, \
             tc.tile_pool(name="score", bufs=2, space="PSUM") as score_ps, \
             tc.tile_pool(name="pvout", bufs=pv_bufs, space="PSUM") as out_ps, \
             tc.tile_pool(name="proj", bufs=1) as proj_pool, \
             tc.tile_pool(name="ystage", bufs=4) as y_pool, \
             tc.tile_pool(name="xw", bufs=1) as xw_pool:
            # ---- resident loads as three wide tiles (e-chunk blocks side
            # by side in the free dim) so each load is ONE big 3D-strided
            # DMA instead of 8 small issues on the sync queue. x still
            # arrives in s-major slices so V/QKV chains start early. ----
            x_all = xw_pool.tile([128, EC * S], xw_dtype, name="x_all")
            wv_all = xw_pool.tile([128, EC * HC * HD], xw_dtype, name="wv_all")
            wqk_all = xw_pool.tile([128, EC * HC * 128], xw_dtype,
                                   name="wqk_all")
            x_sb = [x_all[:, ec * S:(ec + 1) * S] for ec in range(EC)]
            wv_sb = [wv_all[:, ec * HC * HD:(ec + 1) * HC * HD]
                     for ec in range(EC)]
            wqk_sb = [wqk_all[:, ec * HC * 128:(ec + 1) * HC * 128]
                      for ec in range(EC)]

            def _chunked_src(dram, width):
                # (EC*128, width) dram view as [part 128, ec, width]
                ap = dram.ap()
                return bass.AP(
                    tensor=ap.tensor, offset=ap.offset,
                    ap=[[width, 128], [128 * width, EC], [1, width]],
                )

            nc.sync.dma_start(
                out=wv_all.rearrange("p (c f) -> p c f", c=EC),
                in_=_chunked_src(wvT, HC * HD))
            nc.sync.dma_start(
                out=x_all.rearrange("p (c s) -> p c s", c=EC)[:, :, 0:512],
                in_=bass.AP(tensor=xT.ap().tensor, offset=0,
                            ap=[[S, 128], [128 * S, EC], [1, 512]]))
            nc.sync.dma_start(
                out=wqk_all.rearrange("p (c f) -> p c f", c=EC),
                in_=_chunked_src(wqkT, HC * 128))
            for sb in range(1, QC):
                nc.sync.dma_start(
                    out=x_all.rearrange("p (c s) -> p c s", c=EC)[
                        :, :, sb * 512:(sb + 1) * 512],
                    in_=bass.AP(tensor=xT.ap().tensor, offset=sb * 512,
                                ap=[[S, 128], [128 * S, EC], [1, 512]]))
            wproj_sb = []
            for pp in range(NPAIR):
                wt = proj_pool.tile([128, E], low_dt, name=f"wproj_{pp}")
                nc.sync.dma_start(
                    out=wt, in_=wprojT.ap()[pp * 128:(pp + 1) * 128, :]
                )
                wproj_sb.append(wt)

            mask_sb = []
            for mi in range(4):
                mt = small_pool.tile([128, 512], low_dt, name=f"mask_{mi}",
                                     tag=f"mask{mi}", bufs=1)
                nc.sync.dma_start(out=mt, in_=masks.ap()[mi])
                mask_sb.append(mt)

            ones_row = small_pool.tile([1, 64], low_dt if bcast == "pe" else F32,
                                       name="ones_row", tag="ones", bufs=1)
            nc.vector.memset(ones_row, 1.0)

            # ---- phase A: V natural (s, feat) with ones columns ----
            v_sb = []
            for st in range(ST):
                vt = v_pool.tile([128, VW], low_dt, name=f"v_{st}")
                v_sb.append(vt)
            for st in range(ST):
                # rotate psum pools: up to 6 accumulation chains can run
                # while the input DMA stream is still arriving
                vpool, vtag = [(mm_ps, "mmps"), (out_ps, "pvout"),
                               (score_ps, "score")][st % 3]
                psv = vpool.tile([128, HC * HD], F32, name="psv", tag=vtag)
                for ec in range(EC):
                    nc.tensor.matmul(
                        psv,
                        x_sb[ec][:, st * 128:(st + 1) * 128],
                        wv_sb[ec],
                        start=(ec == 0), stop=(ec == EC - 1),
                    )
                vt = v_sb[st]
                # strided copy psum (128, 8, 64) -> v tile (128, 8, 65)[:, :, :64]
                nc.vector.tensor_copy(
                    vt.rearrange("p (h w) -> p h w", h=HC)[:, :, 0:HD],
                    psv.rearrange("p (h d) -> p h d", h=HC),
                )
                nc.vector.memset(
                    vt.rearrange("p (h w) -> p h w", h=HC)[:, :, HD:HD + 1], 1.0
                )

            # ---- per-pair QKV + attention ----
            attn_sb = []
            for pp in range(NPAIR):
                at = attn_pool.tile([128, S], low_dt, name=f"attn_{pp}")
                attn_sb.append(at)

            for pp in range(NPAIR):
                # B1: qkT tiles for this pair (q tile then k tile)
                pair_tiles = []
                for ft in range(2):  # 0 = q-pair, 1 = k-pair
                    qkt = qk_pool.tile([128, S], qk_dtype, name=f"qk_{pp}_{ft}", tag="qk")
                    fcol = pp * 256 + ft * 128
                    for sc2 in range(2):  # LDW amortized over 2 s-chunks
                        pss = [
                            mm_ps.tile([128, 512], F32, name="psqk", tag="mmps")
                            for _ in range(2)
                        ]
                        for ec in range(EC):
                            for k in range(2):
                                sc = sc2 * 2 + k
                                nc.tensor.matmul(
                                    pss[k],
                                    wqk_sb[ec][:, fcol:fcol + 128],
                                    x_sb[ec][:, sc * 512:(sc + 1) * 512],
                                    start=(ec == 0), stop=(ec == EC - 1),
                                )
                        for k in range(2):
                            sc = sc2 * 2 + k
                            eng = qkcopy
                            if qkcopy == "mix":
                                eng = "dve" if (sc2 * 2 + k) % 2 else "act"
                            if eng == "dve":
                                nc.vector.tensor_copy(
                                    qkt[:, sc * 512:(sc + 1) * 512], pss[k]
                                )
                            else:
                                nc.scalar.copy(
                                    qkt[:, sc * 512:(sc + 1) * 512], pss[k]
                                )
                    pair_tiles.append(qkt)
                qt, kt = pair_tiles

                # B2: attention, heads A (rows 0:64) and B (rows 64:128)
                mask_i = 0
                for qc in range(QC):
                    kmax = 4 * qc + 4
                    pso = [
                        out_ps.tile([128, 512], F32, name=f"pso{hh}", tag="pvout")
                        for hh in range(2)
                    ]
                    for kblk in range(kmax):
                        off = max((kblk - 4 * qc) * 128, 0)
                        W = 512 - off  # valid q span [off, 512) of this chunk
                        # scores for both heads into one 2-bank psum tensor
                        pss = score_ps.tile([128, 1024], F32, name="scr", tag="score")
                        pss3 = pss.rearrange("p (t q) -> p t q", t=2)
                        for hh in range(2):
                            lo, hi = hh * 64, hh * 64 + 64
                            nc.tensor.matmul(
                                pss3[:, hh, off:512],
                                kt[lo:hi, kblk * 128:(kblk + 1) * 128],
                                qt[lo:hi, qc * 512 + off:(qc + 1) * 512],
                                start=True, stop=True,
                            )
                        pb = probs_pool.tile(
                            [128, 2, W], low_dt, name="pb", tag="probs"
                        )
                        nc.scalar.activation(
                            out=pb, in_=pss3[:, :, off:512], func=AF.Exp
                        )
                        if (kblk - 4 * qc) * 128 >= 0:
                            # mask the leading (128,128) triangle: keep q'>=k
                            tri = pb[:, :, 0:128]
                            use_pool = (maskeng == "pool") or (
                                maskeng == "mix" and mask_i % 2 == 0)
                            if use_pool:
                                nc.gpsimd.affine_select(
                                    out=tri, in_=tri,
                                    compare_op=AluOpType.is_ge,
                                    fill=0.0, base=0,
                                    pattern=[[0, 2], [1, 128]],
                                    channel_multiplier=-1,
                                )
                            else:
                                mk = mask_sb[0]
                                mk3 = bass.AP(
                                    tensor=mk.tensor, offset=mk.offset,
                                    ap=[mk.ap[0], [0, 2], [1, 128]],
                                )
                                nc.vector.tensor_tensor(
                                    out=tri, in0=tri, in1=mk3,
                                    op=AluOpType.mult,
                                )
                            mask_i += 1
                        for hh in range(2):
                            h_local = pp * 2 + hh
                            vcols = h_local * (HD + 1)
                            nc.tensor.matmul(
                                pso[hh][0:65, off:512],
                                v_sb[kblk][:, vcols:vcols + HD + 1],
                                pb[:, hh, :],
                                start=(kblk == 0), stop=(kblk == kmax - 1),
                            )
                    # normalize: rows 0:64 / row 64
                    for hh in range(2):
                        # bf16 rec_t so the PE broadcast matmul runs at full
                        # rate (f32 would need f32r pre-rounding)
                        rec_dt = low_dt if bcast == "pe" else F32
                        rec_t = small_pool.tile([1, 512], rec_dt, name="rec_t",
                                                tag="rec")
                        if rec == "nr":
                            # table-free reciprocal on DVE: int-magic seed +
                            # one Newton step in the (t-2)*y0 form, which
                            # yields -1/d; compensated by negated wproj on
                            # the host.
                            y0i = small_pool.tile([1, 512], I32, name="y0i",
                                                  tag="y0i")
                            nc.vector.tensor_scalar(
                                out=y0i, in0=pso[hh][64:65, :].bitcast(I32),
                                scalar1=-1, scalar2=RECIP_MAGIC,
                                op0=AluOpType.mult, op1=AluOpType.add)
                            y0 = y0i.bitcast(F32)
                            tprod = small_pool.tile([1, 512], F32, name="tprod",
                                                    tag="tprod")
                            nc.vector.tensor_tensor(
                                out=tprod, in0=pso[hh][64:65, :], in1=y0,
                                op=AluOpType.mult)
                            nc.vector.scalar_tensor_tensor(
                                out=rec_t, in0=tprod, scalar=2.0, in1=y0,
                                op0=AluOpType.subtract, op1=AluOpType.mult)
                        elif rec == "fast":
                            nc.vector.reciprocal_approx_fast(
                                out=rec_t, in_=pso[hh][64:65, :])
                        elif rec == "act":
                            _act_raw(nc, rec_t, pso[hh][64:65, :], AF.Reciprocal)
                        elif rec == "lnexp":
                            # 1/x = exp(-ln x); Ln+Exp share one ACT table
                            # set, unlike Reciprocal (avoids table reloads)
                            lnr = small_pool.tile([1, 512], F32, name="lnr",
                                                  tag="lnr")
                            nc.scalar.activation(out=lnr, in_=pso[hh][64:65, :],
                                                 func=AF.Ln)
                            nc.scalar.activation(out=rec_t, in_=lnr,
                                                 func=AF.Exp, scale=-1.0)
                        else:
                            nc.vector.reciprocal(out=rec_t, in_=pso[hh][64:65, :])
                        if bcast == "pe":
                            # broadcast 1/denom into rows 64:128 of the same
                            # PSUM bank via a rank-1 matmul (row 64 is dead
                            # after the reciprocal read)
                            nc.tensor.matmul(
                                pso[hh][64:128, :], ones_row, rec_t,
                                start=True, stop=True, skip_group_check=True,
                            )
                            rb = pso[hh][64:128, :]
                        else:
                            rb = small_pool.tile([64, 512], F32, name="recb",
                                                 tag="recb")
                            nc.gpsimd.partition_broadcast(rb, rec_t)
                        nc.vector.tensor_tensor(
                            out=attn_sb[pp][hh * 64:hh * 64 + 64,
                                            qc * 512:(qc + 1) * 512],
                            in0=pso[hh][0:64, :], in1=rb, op=AluOpType.mult,
                        )

            # ---- phase C: projection ----
            for qc in range(QC):
                for et in range(EC):
                    psy = mm_ps.tile([128, 512], F32, name="psy", tag="mmps")
                    for pp in range(NPAIR):
                        nc.tensor.matmul(
                            psy,
                            wproj_sb[pp][:, et * 128:(et + 1) * 128],
                            attn_sb[pp][:, qc * 512:(qc + 1) * 512],
                            start=(pp == 0), stop=(pp == NPAIR - 1),
                        )
                    ysb = y_pool.tile([128, 512], F32, name="ysb", tag="y")
                    if ycopy == "dve":
                        nc.vector.tensor_copy(ysb, psy)
                    else:
                        nc.scalar.copy(ysb, psy)
                    nc.sync.dma_start(
                        out=yT.ap()[et * 128:(et + 1) * 128,
                                    qc * 512:(qc + 1) * 512],
                        in_=ysb,
                    )


_NC_CACHE = None


def _get_nc():
    global _NC_CACHE
    if _NC_CACHE is None:
        _NC_CACHE = build_nc()
    return _NC_CACHE


def prepare_in_maps(x, w_qkv, w_proj, b_proj, low_np=None, xw_np=None,
                    negate_proj=True):
    if low_np is None:
        low_np = ml_dtypes.bfloat16
    if xw_np is None:
        xw_np = ml_dtypes.bfloat16
    """Shard + lay out inputs for the 8 cores. Core i = (b=i//2, hg=i%2)."""
    x = np.asarray(x, dtype=np.float32)
    w_qkv = np.asarray(w_qkv, dtype=np.float32)
    w_proj = np.asarray(w_proj, dtype=np.float32)
    scale = float(HD) ** -0.5

    xTs = [np.ascontiguousarray(x[b].T).astype(xw_np) for b in range(B)]

    per_hg = []
    for hg in range(2):
        heads = [hg * HC + i for i in range(HC)]
        qk_rows = []
        v_rows = []
        proj_cols = []
        for p in range(NPAIR):
            hA, hB = heads[2 * p], heads[2 * p + 1]
            qk_rows += list(range(hA * 192, hA * 192 + 64))
            qk_rows += list(range(hB * 192, hB * 192 + 64))
            qk_rows += list(range(hA * 192 + 64, hA * 192 + 128))
            qk_rows += list(range(hB * 192 + 64, hB * 192 + 128))
            v_rows += list(range(hA * 192 + 128, hA * 192 + 192))
            v_rows += list(range(hB * 192 + 128, hB * 192 + 192))
            proj_cols += list(range(hA * 64, hA * 64 + 64))
            proj_cols += list(range(hB * 64, hB * 64 + 64))
        wqk = w_qkv[qk_rows, :].copy()
        # scale q rows (first 128 of every 256-col block -> rows here)
        for p in range(NPAIR):
            wqk[p * 256: p * 256 + 128] *= scale
        wqkT = np.ascontiguousarray(wqk.T).astype(xw_np)
        wvT = np.ascontiguousarray(w_qkv[v_rows, :].T).astype(xw_np)
        wpT = np.ascontiguousarray(w_proj[:, proj_cols].T)
        if negate_proj:  # rec='nr' produces -1/denom; fold the sign here
            wpT = -wpT
        wprojT = wpT.astype(low_np)
        per_hg.append((wqkT, wvT, wprojT))

    k_idx = np.arange(128)[:, None]
    q_idx = np.arange(512)[None, :]
    masks = np.stack([
        (q_idx - k_idx - off * 128 >= 0) for off in range(4)
    ]).astype(low_np)

    in_maps = []
    for i in range(8):
        b, hg = i // 2, i % 2
        wqkT, wvT, wprojT = per_hg[hg]
        in_maps.append({
            "xT": xTs[b],
            "wqkT": wqkT,
            "wvT": wvT,
            "wprojT": wprojT,
            "masks": masks,
        })
    return in_maps


def postprocess(results, b_proj):
    """results: list of 8 dicts with 'yT' (E, S) partials."""
    b_proj = np.asarray(b_proj, dtype=np.float32)
    out = np.empty((B, S, E), dtype=np.float32)
    for b in range(B):
        yT = results[2 * b]["yT"] + results[2 * b + 1]["yT"]
        out[b] = yT.T + b_proj[None, :]
    return out


def run_on_cores(in_maps, trace=False, **kwargs):
    from concourse.bass_utils import run_bass_kernel_spmd
    nc = _get_nc()
    return run_bass_kernel_spmd(nc, in_maps, core_ids=list(range(8)),
                                trace=trace, **kwargs)


def kernel(x, w_qkv, w_proj, b_proj):
    in_maps = prepare_in_maps(x, w_qkv, w_proj, b_proj)
    res = run_on_cores(in_maps)
    return postprocess(res.results, b_proj)



# revision 40
# speedup vs baseline: 1.0041x; 1.0041x over previous
"""Distributed MultiHeadAttention kernel for 8 TRN2 NeuronCores.

Problem: B=4, S=2048, E=1024, H=16 heads of dim 64, causal attention.
Sharding: core i handles (batch b = i//2, head-group hg = i%2) -> 8 heads.
Each core computes qkv for its heads, causal attention, and a partial
output projection over its heads' features; the host sums the two
partials per batch and adds the bias.

Layout notes (per core):
  xT      (1024, 2048) bf16 : x[b].T               (e on partitions)
  wqkT    (1024, 1024) bf16 : per-pair [qA|qB|kA|kB] blocks of 128 cols,
                              q rows pre-scaled by HD**-0.5
  wvT     (1024, 512)  bf16 : v weights, head-major (h*64+d)
  wprojT  (512, 1024)  bf16 : rows c=(pair, head-in-pair, d), cols e
  masks   (4, 128, 512) bf16: causal step masks (only [0] used)
  yT out  (1024, 2048) f32  : partial (W_proj @ attn.T), pre-bias

Inputs are DMA'd interleaved (wv, x s-slice 0, wqk, x s-slices 1-3,
wproj) so the V and QKV accumulation chains start on the first chunks;
V-phase psum chains rotate over all three PSUM pools during the stream.

On-chip pipeline per pair p (heads 2p, 2p+1 packed at partitions 0:64 /
64:128 for K=64 tensor-engine row-group pairing):
  qkT tiles (128, 2048) fp16 produced by bf16 QKV matmuls (DVE/ACT copy),
  scoresT (k-part, q-free) (128, 2x512) blocks in one 2-bank PSUM tensor
  -> single wide exp (ACT, bf16 out), diagonal blocks trimmed to the
     valid causal span; the leading (128,128) triangle masked via
     gpsimd affine_select (or DVE mask-multiply)
  -> PV with per-head [V|1] bf16 stationary -> rows 0:65 of a (128, 512)
     PSUM accum (row 64 = softmax denominator)
  -> per-(qc,head) reciprocal (rec= act | lnexp | vec | fast) + gpsimd
     partition_broadcast (or PE rank-1 matmul into rows 64:128 of the
     same bank) + DVE multiply -> normalized attnT bf16
  proj: wprojT bf16 stationary x attnT -> yT partial blocks (f32 out).
"""

import numpy as np
import ml_dtypes

import concourse.bass as bass
import concourse.mybir as mybir
import concourse.tile as tile
from concourse import bacc
from concourse.alu_op_type import AluOpType

F32 = mybir.dt.float32
F32R = mybir.dt.float32r
BF16 = mybir.dt.bfloat16
F16 = mybir.dt.float16
I32 = mybir.dt.int32
AF = mybir.ActivationFunctionType

# magic constant for the fast-inverse seed y0 = bits^-1(C - bits(x))
RECIP_MAGIC = 0x7EF311C3

B, S, E, H = 4, 2048, 1024, 16
HD = 64
HC = 8           # heads per core
NPAIR = 4        # head pairs per core
EC = E // 128    # 8 e-chunks
QC = S // 512    # 4 q-chunks
KB = S // 128    # 16 k-blocks
ST = S // 128    # 16 s-tiles
VW = HC * (HD + 1)  # 520: v features + per-head ones column


def _act_raw(nc, out, in_, func):
    eng = nc.scalar
    inputs = [eng.lower_ap(in_)]
    for val in (0.0, 1.0, 0.0):  # bias, scale, alpha
        inputs.append(mybir.ImmediateValue(dtype=mybir.dt.float32, value=val))
    return eng.add_instruction(
        mybir.InstActivation(
            name=nc.get_next_instruction_name(),
            func=func,
            ins=inputs,
            outs=[eng.lower_ap(out)],
        )
    )


def build_nc(repeats=1, qk_dtype=F16, low_dt=BF16, probs_bufs=8,
             qk_bufs=4, ycopy="dve", rec="nr", qkcopy="dve", maskeng="pool",
             xw_dtype=BF16, mm_bufs=2, pv_bufs=2, small_bufs=4, bcast="pool"):
    nc = bacc.Bacc("TRN2", target_bir_lowering=False, debug=False)
    xT = nc.dram_tensor("xT", (E, S), xw_dtype, kind="ExternalInput")
    wqkT = nc.dram_tensor("wqkT", (E, HC * 128), xw_dtype, kind="ExternalInput")
    wvT = nc.dram_tensor("wvT", (E, HC * HD), xw_dtype, kind="ExternalInput")
    wprojT = nc.dram_tensor("wprojT", (HC * HD, E), low_dt, kind="ExternalInput")
    masks = nc.dram_tensor("masks", (4, 128, 512), low_dt, kind="ExternalInput")
    yT = nc.dram_tensor("yT", (E, S), F32, kind="ExternalOutput")

    with tile.TileContext(nc) as tc:
        for _rep in range(repeats):
            _emit_body(nc, tc, xT, wqkT, wvT, wprojT, masks, yT,
                       qk_dtype=qk_dtype, low_dt=low_dt,
                       probs_bufs=probs_bufs, qk_bufs=qk_bufs, ycopy=ycopy,
                       rec=rec, qkcopy=qkcopy, maskeng=maskeng,
                       xw_dtype=xw_dtype, mm_bufs=mm_bufs, pv_bufs=pv_bufs,
                       small_bufs=small_bufs, bcast=bcast)
    nc.compile()
    return nc


def _emit_body(nc, tc, xT, wqkT, wvT, wprojT, masks, yT, qk_dtype=F16,
               low_dt=BF16, probs_bufs=8, qk_bufs=4, ycopy="dve",
               rec="nr", qkcopy="dve", maskeng="pool", xw_dtype=BF16,
               mm_bufs=2, pv_bufs=2, small_bufs=4, bcast="pool"):
    if True:
        with tc.tile_pool(name="vp", bufs=1) as v_pool, \
             tc.tile_pool(name="qk", bufs=qk_bufs) as qk_pool, \
             tc.tile_pool(name="probs", bufs=probs_bufs) as probs_pool, \
             tc.tile_pool(name="attn", bufs=1) as attn_pool, \
             tc.tile_pool(name="small", bufs=small_bufs) as small_pool, \
             tc.tile_pool(name="mm", bufs=mm_bufs, space="PSUM") as mm_ps, \
             tc.tile_pool(name="score", bufs=2, space="PSUM") as score_ps, \
             tc.tile_pool(name="pvout", bufs=pv_bufs, space="PSUM") as out_ps, \
             tc.tile_pool(name="proj", bufs=1) as proj_pool, \
             tc.tile_pool(name="ystage", bufs=4) as y_pool, \
             tc.tile_pool(name="xw", bufs=1) as xw_pool:
            # ---- resident loads as three wide tiles (e-chunk blocks side
            # by side in the free dim) so each load is ONE big 3D-strided
            # DMA instead of 8 small issues on the sync queue. x still
            # arrives in s-major slices so V/QKV chains start early. ----
            x_all = xw_pool.tile([128, EC * S], xw_dtype, name="x_all")
            wv_all = xw_pool.tile([128, EC * HC * HD], xw_dtype, name="wv_all")
            wqk_all = xw_pool.tile([128, EC * HC * 128], xw_dtype,
                                   name="wqk_all")
            x_sb = [x_all[:, ec * S:(ec + 1) * S] for ec in range(EC)]
            wv_sb = [wv_all[:, ec * HC * HD:(ec + 1) * HC * HD]
                     for ec in range(EC)]
            wqk_sb = [wqk_all[:, ec * HC * 128:(ec + 1) * HC * 128]
                      for ec in range(EC)]

            def _chunked_src(dram, width):
                # (EC*128, width) dram view as [part 128, ec, width]
                ap = dram.ap()
                return bass.AP(
                    tensor=ap.tensor, offset=ap.offset,
                    ap=[[width, 128], [128 * width, EC], [1, width]],
                )

            nc.sync.dma_start(
                out=wv_all.rearrange("p (c f) -> p c f", c=EC),
                in_=_chunked_src(wvT, HC * HD))
            nc.sync.dma_start(
                out=x_all.rearrange("p (c s) -> p c s", c=EC)[:, :, 0:512],
                in_=bass.AP(tensor=xT.ap().tensor, offset=0,
                            ap=[[S, 128], [128 * S, EC], [1, 512]]))
            nc.sync.dma_start(
                out=wqk_all.rearrange("p (c f) -> p c f", c=EC),
                in_=_chunked_src(wqkT, HC * 128))
            for sb in range(1, QC):
                nc.sync.dma_start(
                    out=x_all.rearrange("p (c s) -> p c s", c=EC)[
                        :, :, sb * 512:(sb + 1) * 512],
                    in_=bass.AP(tensor=xT.ap().tensor, offset=sb * 512,
                                ap=[[S, 128], [128 * S, EC], [1, 512]]))
            wproj_sb = []
            for pp in range(NPAIR):
                wt = proj_pool.tile([128, E], low_dt, name=f"wproj_{pp}")
                nc.sync.dma_start(
                    out=wt, in_=wprojT.ap()[pp * 128:(pp + 1) * 128, :]
                )
                wproj_sb.append(wt)

            mask_sb = []
            for mi in range(4):
                mt = small_pool.tile([128, 512], low_dt, name=f"mask_{mi}",
                                     tag=f"mask{mi}", bufs=1)
                nc.sync.dma_start(out=mt, in_=masks.ap()[mi])
                mask_sb.append(mt)

            ones_row = small_pool.tile([1, 64], low_dt if bcast == "pe" else F32,
                                       name="ones_row", tag="ones", bufs=1)
            nc.vector.memset(ones_row, 1.0)

            # ---- phase A: V natural (s, feat) with ones columns ----
            v_sb = []
            for st in range(ST):
                vt = v_pool.tile([128, VW], low_dt, name=f"v_{st}")
                v_sb.append(vt)
            for st in range(ST):
                # rotate psum pools: up to 6 accumulation chains can run
                # while the input DMA stream is still arriving
                vpool, vtag = [(mm_ps, "mmps"), (out_ps, "pvout"),
                               (score_ps, "score")][st % 3]
                psv = vpool.tile([128, HC * HD], F32, name="psv", tag=vtag)
                for ec in range(EC):
                    nc.tensor.matmul(
                        psv,
                        x_sb[ec][:, st * 128:(st + 1) * 128],
                        wv_sb[ec],
                        start=(ec == 0), stop=(ec == EC - 1),
                    )
                vt = v_sb[st]
                # strided copy psum (128, 8, 64) -> v tile (128, 8, 65)[:, :, :64]
                nc.vector.tensor_copy(
                    vt.rearrange("p (h w) -> p h w", h=HC)[:, :, 0:HD],
                    psv.rearrange("p (h d) -> p h d", h=HC),
                )
                nc.vector.memset(
                    vt.rearrange("p (h w) -> p h w", h=HC)[:, :, HD:HD + 1], 1.0
                )

            # ---- per-pair QKV + attention ----
            attn_sb = []
            for pp in range(NPAIR):
                at = attn_pool.tile([128, S], low_dt, name=f"attn_{pp}")
                attn_sb.append(at)

            for pp in range(NPAIR):
                # B1: qkT tiles for this pair (q tile then k tile)
                pair_tiles = []
                for ft in range(2):  # 0 = q-pair, 1 = k-pair
                    qkt = qk_pool.tile([128, S], qk_dtype, name=f"qk_{pp}_{ft}", tag="qk")
                    fcol = pp * 256 + ft * 128
                    for sc2 in range(2):  # LDW amortized over 2 s-chunks
                        pss = [
                            mm_ps.tile([128, 512], F32, name="psqk", tag="mmps")
                            for _ in range(2)
                        ]
                        for ec in range(EC):
                            for k in range(2):
                                sc = sc2 * 2 + k
                                nc.tensor.matmul(
                                    pss[k],
                                    wqk_sb[ec][:, fcol:fcol + 128],
                                    x_sb[ec][:, sc * 512:(sc + 1) * 512],
                                    start=(ec == 0), stop=(ec == EC - 1),
                                )
                        for k in range(2):
                            sc = sc2 * 2 + k
                            eng = qkcopy
                            if qkcopy == "mix":
                                eng = "dve" if (sc2 * 2 + k) % 2 else "act"
                            if eng == "dve":
                                nc.vector.tensor_copy(
                                    qkt[:, sc * 512:(sc + 1) * 512], pss[k]
                                )
                            else:
                                nc.scalar.copy(
                                    qkt[:, sc * 512:(sc + 1) * 512], pss[k]
                                )
                    pair_tiles.append(qkt)
                qt, kt = pair_tiles

                # B2: attention, heads A (rows 0:64) and B (rows 64:128)
                mask_i = 0
                for qc in range(QC):
                    kmax = 4 * qc + 4
                    pso = [
                        out_ps.tile([128, 512], F32, name=f"pso{hh}", tag="pvout")
                        for hh in range(2)
                    ]
                    for kblk in range(kmax):
                        off = max((kblk - 4 * qc) * 128, 0)
                        W = 512 - off  # valid q span [off, 512) of this chunk
                        # scores for both heads into one 2-bank psum tensor
                        pss = score_ps.tile([128, 1024], F32, name="scr", tag="score")
                        pss3 = pss.rearrange("p (t q) -> p t q", t=2)
                        for hh in range(2):
                            lo, hi = hh * 64, hh * 64 + 64
                            nc.tensor.matmul(
                                pss3[:, hh, off:512],
                                kt[lo:hi, kblk * 128:(kblk + 1) * 128],
                                qt[lo:hi, qc * 512 + off:(qc + 1) * 512],
                                start=True, stop=True,
                            )
                        pb = probs_pool.tile(
                            [128, 2, W], low_dt, name="pb", tag="probs"
                        )
                        nc.scalar.activation(
                            out=pb, in_=pss3[:, :, off:512], func=AF.Exp
                        )
                        if (kblk - 4 * qc) * 128 >= 0:
                            # mask the leading (128,128) triangle: keep q'>=k
                            tri = pb[:, :, 0:128]
                            use_pool = (maskeng == "pool") or (
                                maskeng == "mix" and mask_i % 2 == 0)
                            if use_pool:
                                nc.gpsimd.affine_select(
                                    out=tri, in_=tri,
                                    compare_op=AluOpType.is_ge,
                                    fill=0.0, base=0,
                                    pattern=[[0, 2], [1, 128]],
                                    channel_multiplier=-1,
                                )
                            else:
                                mk = mask_sb[0]
                                mk3 = bass.AP(
                                    tensor=mk.tensor, offset=mk.offset,
                                    ap=[mk.ap[0], [0, 2], [1, 128]],
                                )
                                nc.vector.tensor_tensor(
                                    out=tri, in0=tri, in1=mk3,
                                    op=AluOpType.mult,
                                )
                            mask_i += 1
                        for hh in range(2):
                            h_local = pp * 2 + hh
                            vcols = h_local * (HD + 1)
                            nc.tensor.matmul(
                                pso[hh][0:65, off:512],
                                v_sb[kblk][:, vcols:vcols + HD + 1],
                                pb[:, hh, :],
                                start=(kblk == 0), stop=(kblk == kmax - 1),
                            )
                    # normalize: rows 0:64 / row 64
                    for hh in range(2):
                        # bf16 rec_t so the PE broadcast matmul runs at full
                        # rate (f32 would need f32r pre-rounding)
                        rec_dt = low_dt if bcast == "pe" else F32
                        rec_t = small_pool.tile([1, 512], rec_dt, name="rec_t",
                                                tag="rec")
                        if rec == "nr":
                            # table-free reciprocal on DVE: int-magic seed +
                            # one Newton step in the (t-2)*y0 form, which
                            # yields -1/d; compensated by negated wproj on
                            # the host.
                            y0i = small_pool.tile([1, 512], I32, name="y0i",
                                                  tag="y0i")
                            nc.vector.tensor_scalar(
                                out=y0i, in0=pso[hh][64:65, :].bitcast(I32),
                                scalar1=-1, scalar2=RECIP_MAGIC,
                                op0=AluOpType.mult, op1=AluOpType.add)
                            y0 = y0i.bitcast(F32)
                            tprod = small_pool.tile([1, 512], F32, name="tprod",
                                                    tag="tprod")
                            nc.vector.tensor_tensor(
                                out=tprod, in0=pso[hh][64:65, :], in1=y0,
                                op=AluOpType.mult)
                            nc.vector.scalar_tensor_tensor(
                                out=rec_t, in0=tprod, scalar=2.0, in1=y0,
                                op0=AluOpType.subtract, op1=AluOpType.mult)
                        elif rec == "fast":
                            nc.vector.reciprocal_approx_fast(
                                out=rec_t, in_=pso[hh][64:65, :])
                        elif rec == "act":
                            _act_raw(nc, rec_t, pso[hh][64:65, :], AF.Reciprocal)
                        elif rec == "lnexp":
                            # 1/x = exp(-ln x); Ln+Exp share one ACT table
                            # set, unlike Reciprocal (avoids table reloads)
                            lnr = small_pool.tile([1, 512], F32, name="lnr",
                                                  tag="lnr")
                            nc.scalar.activation(out=lnr, in_=pso[hh][64:65, :],
                                                 func=AF.Ln)
                            nc.scalar.activation(out=rec_t, in_=lnr,
                                                 func=AF.Exp, scale=-1.0)
                        else:
                            nc.vector.reciprocal(out=rec_t, in_=pso[hh][64:65, :])
                        if bcast == "pe":
                            # broadcast 1/denom into rows 64:128 of the same
                            # PSUM bank via a rank-1 matmul (row 64 is dead
                            # after the reciprocal read)
                            nc.tensor.matmul(
                                pso[hh][64:128, :], ones_row, rec_t,
                                start=True, stop=True, skip_group_check=True,
                            )
                            rb = pso[hh][64:128, :]
                        else:
                            rb = small_pool.tile([64, 512], F32, name="recb",
                                                 tag="recb")
                            nc.gpsimd.partition_broadcast(rb, rec_t)
                        nc.vector.tensor_tensor(
                            out=attn_sb[pp][hh * 64:hh * 64 + 64,
                                            qc * 512:(qc + 1) * 512],
                            in0=pso[hh][0:64, :], in1=rb, op=AluOpType.mult,
                        )

            # ---- phase C: projection ----
            for qc in range(QC):
                for et in range(EC):
                    psy = mm_ps.tile([128, 512], F32, name="psy", tag="mmps")
                    for pp in range(NPAIR):
                        nc.tensor.matmul(
                            psy,
                            wproj_sb[pp][:, et * 128:(et + 1) * 128],
                            attn_sb[pp][:, qc * 512:(qc + 1) * 512],
                            start=(pp == 0), stop=(pp == NPAIR - 1),
                        )
                    ysb = y_pool.tile([128, 512], F32, name="ysb", tag="y")
                    if ycopy == "dve":
                        nc.vector.tensor_copy(ysb, psy)
                    else:
                        nc.scalar.copy(ysb, psy)
                    nc.sync.dma_start(
                        out=yT.ap()[et * 128:(et + 1) * 128,
                                    qc * 512:(qc + 1) * 512],
                        in_=ysb,
                    )


_NC_CACHE = None


def _get_nc():
    global _NC_CACHE
    if _NC_CACHE is None:
        _NC_CACHE = build_nc()
    return _NC_CACHE


def prepare_in_maps(x, w_qkv, w_proj, b_proj, low_np=None, xw_np=None,
                    negate_proj=True):
    if low_np is None:
        low_np = ml_dtypes.bfloat16
    if xw_np is None:
        xw_np = ml_dtypes.bfloat16
    """Shard + lay out inputs for the 8 cores. Core i = (b=i//2, hg=i%2)."""
    x = np.asarray(x, dtype=np.float32)
    w_qkv = np.asarray(w_qkv, dtype=np.float32)
    w_proj = np.asarray(w_proj, dtype=np.float32)
    scale = float(HD) ** -0.5

    xTs = [np.ascontiguousarray(x[b].T).astype(xw_np) for b in range(B)]

    per_hg = []
    for hg in range(2):
        heads = [hg * HC + i for i in range(HC)]
        qk_rows = []
        v_rows = []
        proj_cols = []
        for p in range(NPAIR):
            hA, hB = heads[2 * p], heads[2 * p + 1]
            qk_rows += list(range(hA * 192, hA * 192 + 64))
            qk_rows += list(range(hB * 192, hB * 192 + 64))
            qk_rows += list(range(hA * 192 + 64, hA * 192 + 128))
            qk_rows += list(range(hB * 192 + 64, hB * 192 + 128))
            v_rows += list(range(hA * 192 + 128, hA * 192 + 192))
            v_rows += list(range(hB * 192 + 128, hB * 192 + 192))
            proj_cols += list(range(hA * 64, hA * 64 + 64))
            proj_cols += list(range(hB * 64, hB * 64 + 64))
        wqk = w_qkv[qk_rows, :].copy()
        # scale q rows (first 128 of every 256-col block -> rows here)
        for p in range(NPAIR):
            wqk[p * 256: p * 256 + 128] *= scale
        wqkT = np.ascontiguousarray(wqk.T).astype(xw_np)
        wvT = np.ascontiguousarray(w_qkv[v_rows, :].T).astype(xw_np)
        wpT = np.ascontiguousarray(w_proj[:, proj_cols].T)
        if negate_proj:  # rec='nr' produces -1/denom; fold the sign here
            wpT = -wpT
        wprojT = wpT.astype(low_np)
        per_hg.append((wqkT, wvT, wprojT))

    k_idx = np.arange(128)[:, None]
    q_idx = np.arange(512)[None, :]
    masks = np.stack([
        (q_idx - k_idx - off * 128 >= 0) for off in range(4)
    ]).astype(low_np)

    in_maps = []
    for i in range(8):
        b, hg = i // 2, i % 2
        wqkT, wvT, wprojT = per_hg[hg]
        in_maps.append({
            "xT": xTs[b],
            "wqkT": wqkT,
            "wvT": wvT,
            "wprojT": wprojT,
            "masks": masks,
        })
    return in_maps


def postprocess(results, b_proj):
    """results: list of 8 dicts with 'yT' (E, S) partials."""
    b_proj = np.asarray(b_proj, dtype=np.float32)
    out = np.empty((B, S, E), dtype=np.float32)
    for b in range(B):
        yT = results[2 * b]["yT"] + results[2 * b + 1]["yT"]
        out[b] = yT.T + b_proj[None, :]
    return out


def run_on_cores(in_maps, trace=False, **kwargs):
    from concourse.bass_utils import run_bass_kernel_spmd
    nc = _get_nc()
    return run_bass_kernel_spmd(nc, in_maps, core_ids=list(range(8)),
                                trace=trace, **kwargs)


def kernel(x, w_qkv, w_proj, b_proj):
    in_maps = prepare_in_maps(x, w_qkv, w_proj, b_proj)
    res = run_on_cores(in_maps)
    return postprocess(res.results, b_proj)

